# revision 24
# baseline (speedup 1.0000x reference)
"""Trainium2 Bass kernel for EventDiffusion GNN (GCNConv + GATConv, 2 layers).

Dense block-push formulation (no per-edge gathers, no Q7 descriptor storms):

  - nodes padded to NPAD=10240; dst range sharded 8 ways (1280 dst/core);
    src dimension is global (10240) on every core.
  - Layer 1 (GCN): per src-block g (80 blocks of 128), compute
    xw1_g = X_g @ W1 on the fly, then push
       psumT[feat, dst_local] += xw1_g^T @ m1_g
    where m1_g[src_slot, dst_local] is a host-precomputed dense bf16 matrix of
    summed GCN coefficients (zero where no edge).  Output lands transposed
    (H^T), which is exactly the lhsT layout needed by layer 2.
  - Layer 2 (GAT): attention logits are separable: e[s,d] =
    leakyrelu(ssrc[s] + sdst[d]) for (s,d) edges.  Per src block:
       T = (B_g + ssrc_g) + sdst_bcast          (B_g = log(edge count), -3e4 if none)
       A = exp(max(T, 0.2T) - C)                (C=10 constant shift; cancels in softmax)
       psumT[feat, dst] += t2_g^T @ A ; den += ones^T @ A
    Multi-edges are handled exactly: exp(log(count) + e) = count * exp(e).
  - softmax normalization: out = relu(psumT * (1/den) + b2), written transposed;
    host transposes back.
  - one AllGather of the 260-col layer-2 node table (features + ssrc + sdst).
"""

import numpy as np

import concourse.bass as bass
import concourse.bacc as bacc
import concourse.mybir as mybir
import concourse.tile as tile
from concourse.bass_utils import run_bass_kernel_spmd

FP32 = mybir.dt.float32
BF16 = mybir.dt.bfloat16
BF16NP = mybir.dt.np(mybir.dt.bfloat16)

N_CORES = 8
D = 256
T2C = 260          # layer-2 table cols: 256 feats | 256 ssrc | 257 sdst | pad
CSHIFT = 0.0       # constant softmax shift (cancels exactly in the ratio);
                   # logits for this distribution are <1, exp overflows only
                   # past ~85, and a nonzero shift stored in bf16 would cost
                   # ~1.6% relative noise on every attention weight
NEGINF = -30000.0  # log-count placeholder for non-edges

ADD = mybir.AluOpType.add
MUL = mybir.AluOpType.mult
MAX = mybir.AluOpType.max
AF = mybir.ActivationFunctionType


def _bf16(a):
    return np.ascontiguousarray(np.asarray(a, np.float32)).astype(BF16NP)


def _pad_nodes(n):
    return -(-n // (128 * N_CORES)) * (128 * N_CORES)


# ----------------------------------------------------------------------------
# host-side preprocessing
# ----------------------------------------------------------------------------

def _prep(event_emb, edge_index, W1, b1, W2, att_src, att_dst, b2):
    X = np.asarray(event_emb, np.float32)
    n = X.shape[0]
    npad = _pad_nodes(n)
    per = npad // N_CORES
    ngb = npad // 128

    ei = np.asarray(edge_index, np.int64)
    src = np.concatenate([ei[0], np.arange(n, dtype=np.int64)])
    dst = np.concatenate([ei[1], np.arange(n, dtype=np.int64)])
    deg = np.bincount(dst, minlength=n).astype(np.float32)
    dinv = np.where(deg > 0, 1.0 / np.sqrt(deg), 0.0).astype(np.float32)
    coeff = (dinv[src] * dinv[dst]).astype(np.float32)

    core_of = dst // per
    per_core = []
    for c in range(N_CORES):
        m = core_of == c
        s, d = src[m], dst[m] - c * per
        co = coeff[m]
        flat = s * per + d
        m1 = np.zeros(npad * per, np.float32)
        np.add.at(m1, flat, co)
        cnt = np.zeros(npad * per, np.float32)
        np.add.at(cnt, flat, 1.0)
        b2m = np.full(npad * per, NEGINF, np.float32)
        nz = cnt > 0
        b2m[nz] = np.log(cnt[nz]) - CSHIFT
        per_core.append(
            dict(
                m1s=_bf16(m1.reshape(ngb, 128, per)),
                b2s=_bf16(b2m.reshape(ngb, 128, per)),
            )
        )
        del m1, cnt, b2m

    W1 = np.asarray(W1, np.float32)
    W2 = np.asarray(W2, np.float32)
    v1 = (W2 @ np.asarray(att_src, np.float32)).astype(np.float32)
    v2 = (W2 @ np.asarray(att_dst, np.float32)).astype(np.float32)

    Xp = np.zeros((npad, D), np.float32)
    Xp[:n] = X
    W2p = np.zeros((D, T2C), np.float32)
    W2p[:, :D] = W2
    W2p[:, 256] = v1
    W2p[:, 257] = v2

    shared = dict(
        xtb=_bf16(Xp.T.reshape(2, 128, npad)),
        w1b=_bf16(W1.reshape(2, 128, D)),
        w2p=_bf16(W2p.reshape(2, 128, T2C)),
        b1T=np.ascontiguousarray(np.asarray(b1, np.float32).reshape(2, 128).T),
        b2T=np.ascontiguousarray(np.asarray(b2, np.float32).reshape(2, 128).T),
        ones128=_bf16(np.ones((128, 1), np.float32)),
    )
    return shared, per_core, n, npad, per, ngb


# ----------------------------------------------------------------------------
# device program
# ----------------------------------------------------------------------------

def _build_nc(npad):
    per = npad // N_CORES
    ngb = npad // 128
    nblk = per // 128
    # dst column chunks per feature half: psum banks are 512 fp32 wide
    CH = [(0, 512), (512, 1024), (1024, 1280)]
    assert per == 1280

    nc = bacc.Bacc(
        "TRN2", target_bir_lowering=False, debug=False, num_devices=N_CORES
    )

    xtb_d = nc.dram_tensor("xtb", [2, 128, npad], BF16, kind="ExternalInput")
    w1_d = nc.dram_tensor("w1b", [2, 128, D], BF16, kind="ExternalInput")
    w2_d = nc.dram_tensor("w2p", [2, 128, T2C], BF16, kind="ExternalInput")
    b1_d = nc.dram_tensor("b1T", [128, 2], FP32, kind="ExternalInput")
    b2_d = nc.dram_tensor("b2T", [128, 2], FP32, kind="ExternalInput")
    ones_d = nc.dram_tensor("ones128", [128, 1], BF16, kind="ExternalInput")
    m1_d = nc.dram_tensor("m1s", [ngb, 128, per], BF16, kind="ExternalInput")
    b2s_d = nc.dram_tensor("b2s", [ngb, 128, per], BF16, kind="ExternalInput")
    outT_d = nc.dram_tensor("outT", [2, 128, per], FP32, kind="ExternalOutput")

    t2slice = nc.dram_tensor("t2slice", [nblk, 128, T2C], BF16)
    sdst_dram = nc.dram_tensor("sdstd", [per, 1], BF16)
    t2full = nc.dram_tensor(
        "t2full", [N_CORES, nblk, 128, T2C], BF16, addr_space="Shared"
    )

    with tile.TileContext(nc) as tc:
        with tc.tile_pool(name="const", bufs=1) as cp:
            w1_sb = cp.tile([128, 2, D], BF16)
            w2_sb = cp.tile([128, 2, T2C], BF16)
            for k in range(2):
                nc.sync.dma_start(w1_sb[:, k, :], w1_d[k])
                nc.sync.dma_start(w2_sb[:, k, :], w2_d[k])
            b1_sb = cp.tile([128, 2], FP32)
            nc.sync.dma_start(b1_sb[:], b1_d[:, :])
            b2_sb = cp.tile([128, 2], FP32)
            nc.sync.dma_start(b2_sb[:], b2_d[:, :])
            ones_sb = cp.tile([128, 1], BF16)
            nc.sync.dma_start(ones_sb[:], ones_d[:, :])
            ht_sb = cp.tile([128, 2, per], BF16)

            # ---------------- phase 1: GCN (fused XW1 + push) ----------------
            with (
                tc.tile_pool(name="xt_p", bufs=1) as xp,
                tc.tile_pool(name="m1_p", bufs=3) as mp,
                tc.tile_pool(name="xw1_p", bufs=4) as wp,
                tc.psum_pool(name="ps1a_p", bufs=2) as pa,
                tc.psum_pool(name="psT1_p", bufs=1) as pt,
            ):
                xt_sb = xp.tile([128, 2, npad], BF16)
                for k in range(2):
                    nc.sync.dma_start(xt_sb[:, k, :], xtb_d[k])
                # psumT tiles: h0 -> TA,TB,TC[:, :256]; h1 -> TD,TE,TC[:,256:]
                TA = pt.tile([128, 512], FP32)
                TB = pt.tile([128, 512], FP32)
                TD = pt.tile([128, 512], FP32)
                TE = pt.tile([128, 512], FP32)
                TC_ = pt.tile([128, 512], FP32)

                def t1_dst(h, ci):
                    if ci < 2:
                        t = (TA, TB)[ci] if h == 0 else (TD, TE)[ci]
                        return t[:, :]
                    return TC_[:, 0:256] if h == 0 else TC_[:, 256:512]

                for g in range(ngb):
                    m1g = mp.tile([128, per], BF16, tag="m1")
                    nc.sync.dma_start(m1g[:], m1_d[g])
                    ps = pa.tile([128, D], FP32, tag="ps1a")
                    for k in range(2):
                        nc.tensor.matmul(
                            ps[:],
                            lhsT=xt_sb[:, k, g * 128:(g + 1) * 128],
                            rhs=w1_sb[:, k, :],
                            start=(k == 0),
                            stop=(k == 1),
                        )
                    xg = wp.tile([128, D], BF16, tag="xw1")
                    nc.scalar.activation(xg[:], ps[:], AF.Copy)
                    st, sp = (g == 0), (g == ngb - 1)
                    for h in range(2):
                        for ci, (c0, c1) in enumerate(CH):
                            # TC_ holds two accumulation groups in one PSUM
                            # bank; start=True clears the WHOLE bank, so only
                            # the first-issued group (h0) may set it.  The h1
                            # group overwrites its freshly-cleared region via
                            # the per-element has_written bits.
                            nc.tensor.matmul(
                                t1_dst(h, ci),
                                lhsT=xg[:, h * 128:(h + 1) * 128],
                                rhs=m1g[:, c0:c1],
                                start=st and not (h == 1 and ci == 2),
                                stop=sp,
                            )
                # H = relu(aggT + b1), stored transposed bf16
                for h in range(2):
                    for ci, (c0, c1) in enumerate(CH):
                        nc.vector.tensor_scalar(
                            ht_sb[:, h, c0:c1],
                            t1_dst(h, ci),
                            b1_sb[:, h:h + 1],
                            0.0,
                            op0=ADD,
                            op1=MAX,
                        )

            # ---------------- phase 2A: local table2 slice -------------------
            with (
                tc.psum_pool(name="ps2_p", bufs=2) as p2,
                tc.tile_pool(name="st2_p", bufs=3) as s2,
            ):
                for b in range(nblk):
                    ps2t = p2.tile([128, T2C], FP32, tag="ps2")
                    for k in range(2):
                        nc.tensor.matmul(
                            ps2t[:],
                            lhsT=ht_sb[:, k, b * 128:(b + 1) * 128],
                            rhs=w2_sb[:, k, :],
                            start=(k == 0),
                            stop=(k == 1),
                        )
                    st2t = s2.tile([128, T2C], BF16, tag="st2")
                    nc.scalar.activation(st2t[:], ps2t[:], AF.Copy)
                    nc.sync.dma_start(t2slice[b], st2t[:])
                    nc.sync.dma_start(
                        sdst_dram[b * 128:(b + 1) * 128, :], st2t[:, 257:258]
                    )

            nc.gpsimd.collective_compute(
                "AllGather",
                mybir.AluOpType.bypass,
                replica_groups=[list(range(N_CORES))],
                ins=[t2slice[:, :, :]],
                outs=[t2full[:, :, :, :]],
            )

            # ---------------- phase 2B: GAT dense push -----------------------
            with (
                tc.tile_pool(name="t2_p", bufs=1) as tp2,
                tc.tile_pool(name="row_p", bufs=1) as rp,
                tc.tile_pool(name="bc_p", bufs=1) as bcp,
                tc.tile_pool(name="b2g_p", bufs=4) as bp,
                tc.tile_pool(name="T_p", bufs=2) as Tp,
                tc.tile_pool(name="H_p", bufs=2) as Hp,
                tc.tile_pool(name="L_p", bufs=2) as Lp,
                tc.tile_pool(name="A_p", bufs=3) as Ap,
                tc.psum_pool(name="ps2b_p", bufs=1) as pb,
                tc.tile_pool(name="fin_p", bufs=2) as fp_,
            ):
                t2_sb = tp2.tile([128, ngb, T2C], BF16)
                # fp32 copy of the ssrc logit columns (tensor_scalar needs
                # fp32 per-partition scalars)
                ssrc_f32 = bcp.tile([128, ngb], FP32)
                for r in range(N_CORES):
                    nc.sync.dma_start(
                        t2_sb[:, r * nblk:(r + 1) * nblk, :],
                        t2full[r].rearrange("b p c -> p b c"),
                    )
                    nc.vector.tensor_copy(
                        ssrc_f32[:, r * nblk:(r + 1) * nblk],
                        t2_sb[:, r * nblk:(r + 1) * nblk, 256:257],
                    )
                sdstrow = rp.tile([1, per], BF16)
                nc.sync.dma_start(sdstrow[:], sdst_dram[:, :])
                sdst_bc = bcp.tile([128, per], BF16)
                nc.gpsimd.partition_broadcast(sdst_bc[:], sdstrow[:])

                PA = pb.tile([128, 512], FP32)
                PB = pb.tile([128, 512], FP32)
                PD = pb.tile([128, 512], FP32)
                PE_ = pb.tile([128, 512], FP32)
                PC_ = pb.tile([128, 512], FP32)
                DN0 = pb.tile([128, 512], FP32)
                DN1 = pb.tile([128, 512], FP32)
                DN2 = pb.tile([128, 512], FP32)

                def t2_dst(h, ci):
                    if ci < 2:
                        t = (PA, PB)[ci] if h == 0 else (PD, PE_)[ci]
                        return t[:, :]
                    return PC_[:, 0:256] if h == 0 else PC_[:, 256:512]

                dn = [DN0[0:1, :], DN1[0:1, :], DN2[0:1, 0:256]]

                for g in range(ngb):
                    bg = bp.tile([128, per], BF16, tag="b2g")
                    nc.sync.dma_start(bg[:], b2s_d[g])
                    # T = ssrc_g + sdst ; L = leakyrelu(T) = max(T, 0.2T)
                    # L2 = L + (log(count) - C | -inf) ; A = exp(L2)
                    Tt = Tp.tile([128, per], BF16, tag="T")
                    nc.vector.tensor_scalar_add(
                        Tt[:], sdst_bc[:], ssrc_f32[:, g:g + 1]
                    )
                    # leakyrelu: every 3rd block on the scalar engine (Lrelu
                    # lives in the same ACT table set as Exp), rest on DVE
                    Lt = Lp.tile([128, per], BF16, tag="L")
                    if g % 3 == 0:
                        nc.scalar.activation(Lt[:], Tt[:], AF.Lrelu, alpha=0.2)
                    else:
                        nc.vector.scalar_tensor_tensor(
                            Lt[:], Tt[:], 0.2, Tt[:], op0=MUL, op1=MAX
                        )
                    L2 = Lp.tile([128, per], BF16, tag="L2")
                    nc.vector.tensor_tensor(L2[:], Lt[:], bg[:], op=ADD)
                    At = Ap.tile([128, per], BF16, tag="A")
                    nc.scalar.activation(At[:], L2[:], AF.Exp)
                    st, sp = (g == 0), (g == ngb - 1)
                    for h in range(2):
                        for ci, (c0, c1) in enumerate(CH):
                            # PC_ bank shared by h0/h1 chunk-2 groups: only
                            # the h0 group may issue the bank-clearing start.
                            nc.tensor.matmul(
                                t2_dst(h, ci),
                                lhsT=t2_sb[:, g, h * 128:(h + 1) * 128],
                                rhs=At[:, c0:c1],
                                start=st and not (h == 1 and ci == 2),
                                stop=sp,
                            )
                    for ci, (c0, c1) in enumerate(CH):
                        nc.tensor.matmul(
                            dn[ci],
                            lhsT=ones_sb[:],
                            rhs=At[:, c0:c1],
                            start=st,
                            stop=sp,
                        )

                # ---- normalize + bias + relu, write transposed --------------
                denrow = rp.tile([1, per], FP32)
                for ci, (c0, c1) in enumerate(CH):
                    nc.vector.tensor_copy(denrow[:, c0:c1], dn[ci])
                den_bc = bcp.tile([128, per], FP32)
                nc.gpsimd.partition_broadcast(den_bc[:], denrow[:])
                rden = bcp.tile([128, per], FP32)
                nc.vector.reciprocal(rden[:], den_bc[:])

                for h in range(2):
                    for ci, (c0, c1) in enumerate(CH):
                        csz = c1 - c0
                        tmp = fp_.tile([128, 512], FP32, tag="tmp")
                        nc.vector.tensor_tensor(
                            tmp[:, 0:csz], t2_dst(h, ci), rden[:, c0:c1],
                            op=MUL,
                        )
                        oc = fp_.tile([128, 512], FP32, tag="oc")
                        nc.vector.tensor_scalar(
                            oc[:, 0:csz], tmp[:, 0:csz], b2_sb[:, h:h + 1],
                            0.0, op0=ADD, op1=MAX,
                        )
                        nc.sync.dma_start(outT_d[h, :, c0:c1], oc[:, 0:csz])

    nc.finalize()
    return nc


# ----------------------------------------------------------------------------
# entry point
# ----------------------------------------------------------------------------

_CACHE = {}


def _get_nc(npad):
    if npad not in _CACHE:
        _CACHE[npad] = _build_nc(npad)
    return _CACHE[npad]


def kernel(event_emb, edge_index, W1, b1, W2, att_src, att_dst, b2,
           _want_results=False, _trace=False):
    shared, per_core, n, npad, per, ngb = _prep(
        event_emb, edge_index, W1, b1, W2, att_src, att_dst, b2
    )
    nc = _get_nc(npad)
    in_maps = [{**shared, **per_core[c]} for c in range(N_CORES)]
    res = run_bass_kernel_spmd(
        nc, in_maps, core_ids=list(range(N_CORES)), trace=_trace
    )
    outs = []
    for c in range(N_CORES):
        oT = np.asarray(res.results[c]["outT"], np.float32)  # [2,128,per]
        outs.append(oT.reshape(D, per).T)  # [per, D]
    out = np.concatenate(outs, axis=0)[:n]
    if _want_results:
        return out, res
    return out


# revision 25
# speedup vs baseline: 1.0704x; 1.0704x over previous
"""Trainium2 Bass kernel for EventDiffusion GNN (GCNConv + GATConv, 2 layers).

Dense block-push formulation (no per-edge gathers, no Q7 descriptor storms):

  - nodes padded to NPAD=10240; dst range sharded 8 ways (1280 dst/core);
    src dimension is global (10240) on every core.
  - Layer 1 (GCN): per src-block g (80 blocks of 128), compute
    xw1_g = X_g @ W1 on the fly, then push
       psumT[feat, dst_local] += xw1_g^T @ m1_g
    where m1_g[src_slot, dst_local] is a host-precomputed dense bf16 matrix of
    summed GCN coefficients (zero where no edge).  Output lands transposed
    (H^T), which is exactly the lhsT layout needed by layer 2.
  - Layer 2 (GAT): attention logits are separable: e[s,d] =
    leakyrelu(ssrc[s] + sdst[d]) for (s,d) edges.  Per src block:
       T = (B_g + ssrc_g) + sdst_bcast          (B_g = log(edge count), -3e4 if none)
       A = exp(max(T, 0.2T) - C)                (C=10 constant shift; cancels in softmax)
       psumT[feat, dst] += t2_g^T @ A ; den += ones^T @ A
    Multi-edges are handled exactly: exp(log(count) + e) = count * exp(e).
  - softmax normalization: out = relu(psumT * (1/den) + b2), written transposed;
    host transposes back.
  - one AllGather of the 260-col layer-2 node table (features + ssrc + sdst).
"""

import numpy as np

import concourse.bass as bass
import concourse.bacc as bacc
import concourse.mybir as mybir
import concourse.tile as tile
from concourse.bass_utils import run_bass_kernel_spmd

FP32 = mybir.dt.float32
BF16 = mybir.dt.bfloat16
BF16NP = mybir.dt.np(mybir.dt.bfloat16)

N_CORES = 8
D = 256
T2C = 260          # layer-2 table cols: 256 feats | 256 ssrc | 257 sdst | pad
CSHIFT = 0.0       # constant softmax shift (cancels exactly in the ratio);
                   # logits for this distribution are <1, exp overflows only
                   # past ~85, and a nonzero shift stored in bf16 would cost
                   # ~1.6% relative noise on every attention weight
NEGINF = -30000.0  # log-count placeholder for non-edges

ADD = mybir.AluOpType.add
MUL = mybir.AluOpType.mult
MAX = mybir.AluOpType.max
AF = mybir.ActivationFunctionType


def _bf16(a):
    return np.ascontiguousarray(np.asarray(a, np.float32)).astype(BF16NP)


def _pad_nodes(n):
    return -(-n // (128 * N_CORES)) * (128 * N_CORES)


# ----------------------------------------------------------------------------
# host-side preprocessing
# ----------------------------------------------------------------------------

def _prep(event_emb, edge_index, W1, b1, W2, att_src, att_dst, b2):
    X = np.asarray(event_emb, np.float32)
    n = X.shape[0]
    npad = _pad_nodes(n)
    per = npad // N_CORES
    ngb = npad // 128

    ei = np.asarray(edge_index, np.int64)
    src = np.concatenate([ei[0], np.arange(n, dtype=np.int64)])
    dst = np.concatenate([ei[1], np.arange(n, dtype=np.int64)])
    deg = np.bincount(dst, minlength=n).astype(np.float32)
    dinv = np.where(deg > 0, 1.0 / np.sqrt(deg), 0.0).astype(np.float32)
    coeff = (dinv[src] * dinv[dst]).astype(np.float32)

    core_of = dst // per
    per_core = []
    for c in range(N_CORES):
        m = core_of == c
        s, d = src[m], dst[m] - c * per
        co = coeff[m]
        flat = s * per + d
        m1 = np.zeros(npad * per, np.float32)
        np.add.at(m1, flat, co)
        cnt = np.zeros(npad * per, np.float32)
        np.add.at(cnt, flat, 1.0)
        b2m = np.full(npad * per, NEGINF, np.float32)
        nz = cnt > 0
        b2m[nz] = np.log(cnt[nz]) - CSHIFT
        per_core.append(
            dict(
                m1s=_bf16(m1.reshape(ngb, 128, per)),
                b2s=_bf16(b2m.reshape(ngb, 128, per)),
            )
        )
        del m1, cnt, b2m

    W1 = np.asarray(W1, np.float32)
    W2 = np.asarray(W2, np.float32)
    v1 = (W2 @ np.asarray(att_src, np.float32)).astype(np.float32)
    v2 = (W2 @ np.asarray(att_dst, np.float32)).astype(np.float32)

    Xp = np.zeros((npad, D), np.float32)
    Xp[:n] = X
    W2p = np.zeros((D, T2C), np.float32)
    W2p[:, :D] = W2
    W2p[:, 256] = v1
    W2p[:, 257] = v2

    shared = dict(
        xtb=_bf16(Xp.T.reshape(2, 128, npad)),
        w1b=_bf16(W1.reshape(2, 128, D)),
        w2p=_bf16(W2p.reshape(2, 128, T2C)),
        b1T=np.ascontiguousarray(np.asarray(b1, np.float32).reshape(2, 128).T),
        b2T=np.ascontiguousarray(np.asarray(b2, np.float32).reshape(2, 128).T),
        ones128=_bf16(np.ones((128, 1), np.float32)),
    )
    return shared, per_core, n, npad, per, ngb


# ----------------------------------------------------------------------------
# device program
# ----------------------------------------------------------------------------

def _build_nc(npad):
    per = npad // N_CORES
    ngb = npad // 128
    nblk = per // 128
    # dst column chunks per feature half: psum banks are 512 fp32 wide
    CH = [(0, 512), (512, 1024), (1024, 1280)]
    assert per == 1280

    nc = bacc.Bacc(
        "TRN2", target_bir_lowering=False, debug=False, num_devices=N_CORES
    )

    xtb_d = nc.dram_tensor("xtb", [2, 128, npad], BF16, kind="ExternalInput")
    w1_d = nc.dram_tensor("w1b", [2, 128, D], BF16, kind="ExternalInput")
    w2_d = nc.dram_tensor("w2p", [2, 128, T2C], BF16, kind="ExternalInput")
    b1_d = nc.dram_tensor("b1T", [128, 2], FP32, kind="ExternalInput")
    b2_d = nc.dram_tensor("b2T", [128, 2], FP32, kind="ExternalInput")
    ones_d = nc.dram_tensor("ones128", [128, 1], BF16, kind="ExternalInput")
    m1_d = nc.dram_tensor("m1s", [ngb, 128, per], BF16, kind="ExternalInput")
    b2s_d = nc.dram_tensor("b2s", [ngb, 128, per], BF16, kind="ExternalInput")
    outT_d = nc.dram_tensor("outT", [2, 128, per], FP32, kind="ExternalOutput")

    t2slice = nc.dram_tensor("t2slice", [nblk, 128, T2C], BF16)
    sdst_dram = nc.dram_tensor("sdstd", [per, 1], BF16)
    t2full = nc.dram_tensor(
        "t2full", [N_CORES, nblk, 128, T2C], BF16, addr_space="Shared"
    )

    with tile.TileContext(nc) as tc:
        with tc.tile_pool(name="const", bufs=1) as cp:
            w1_sb = cp.tile([128, 2, D], BF16)
            w2_sb = cp.tile([128, 2, T2C], BF16)
            for k in range(2):
                nc.sync.dma_start(w1_sb[:, k, :], w1_d[k])
                nc.sync.dma_start(w2_sb[:, k, :], w2_d[k])
            b1_sb = cp.tile([128, 2], FP32)
            nc.sync.dma_start(b1_sb[:], b1_d[:, :])
            b2_sb = cp.tile([128, 2], FP32)
            nc.sync.dma_start(b2_sb[:], b2_d[:, :])
            ones_sb = cp.tile([128, 1], BF16)
            nc.sync.dma_start(ones_sb[:], ones_d[:, :])
            ht_sb = cp.tile([128, 2, per], BF16)

            # ---------------- phase 1: GCN (fused XW1 + push) ----------------
            with (
                tc.tile_pool(name="xt_p", bufs=1) as xp,
                tc.tile_pool(name="m1_p", bufs=3) as mp,
                tc.tile_pool(name="xw1_p", bufs=4) as wp,
                tc.psum_pool(name="ps1a_p", bufs=2) as pa,
                tc.psum_pool(name="psT1_p", bufs=1) as pt,
            ):
                xt_sb = xp.tile([128, 2, npad], BF16)
                for k in range(2):
                    nc.sync.dma_start(xt_sb[:, k, :], xtb_d[k])
                # psumT tiles: h0 -> TA,TB,TC[:, :256]; h1 -> TD,TE,TC[:,256:]
                TA = pt.tile([128, 512], FP32)
                TB = pt.tile([128, 512], FP32)
                TD = pt.tile([128, 512], FP32)
                TE = pt.tile([128, 512], FP32)
                TC_ = pt.tile([128, 512], FP32)

                def t1_dst(h, ci):
                    if ci < 2:
                        t = (TA, TB)[ci] if h == 0 else (TD, TE)[ci]
                        return t[:, :]
                    return TC_[:, 0:256] if h == 0 else TC_[:, 256:512]

                for g in range(ngb):
                    m1g = mp.tile([128, per], BF16, tag="m1")
                    nc.sync.dma_start(m1g[:], m1_d[g])
                    ps = pa.tile([128, D], FP32, tag="ps1a")
                    for k in range(2):
                        nc.tensor.matmul(
                            ps[:],
                            lhsT=xt_sb[:, k, g * 128:(g + 1) * 128],
                            rhs=w1_sb[:, k, :],
                            start=(k == 0),
                            stop=(k == 1),
                        )
                    xg = wp.tile([128, D], BF16, tag="xw1")
                    nc.scalar.activation(xg[:], ps[:], AF.Copy)
                    st, sp = (g == 0), (g == ngb - 1)
                    for h in range(2):
                        for ci, (c0, c1) in enumerate(CH):
                            # TC_ holds two accumulation groups in one PSUM
                            # bank; start=True clears the WHOLE bank, so only
                            # the first-issued group (h0) may set it.  The h1
                            # group overwrites its freshly-cleared region via
                            # the per-element has_written bits.
                            nc.tensor.matmul(
                                t1_dst(h, ci),
                                lhsT=xg[:, h * 128:(h + 1) * 128],
                                rhs=m1g[:, c0:c1],
                                start=st and not (h == 1 and ci == 2),
                                stop=sp,
                            )
                # H = relu(aggT + b1), stored transposed bf16
                for h in range(2):
                    for ci, (c0, c1) in enumerate(CH):
                        nc.vector.tensor_scalar(
                            ht_sb[:, h, c0:c1],
                            t1_dst(h, ci),
                            b1_sb[:, h:h + 1],
                            0.0,
                            op0=ADD,
                            op1=MAX,
                        )

            # ---------------- phase 2A: local table2 slice -------------------
            with (
                tc.psum_pool(name="ps2_p", bufs=2) as p2,
                tc.tile_pool(name="st2_p", bufs=3) as s2,
            ):
                for b in range(nblk):
                    ps2t = p2.tile([128, T2C], FP32, tag="ps2")
                    for k in range(2):
                        nc.tensor.matmul(
                            ps2t[:],
                            lhsT=ht_sb[:, k, b * 128:(b + 1) * 128],
                            rhs=w2_sb[:, k, :],
                            start=(k == 0),
                            stop=(k == 1),
                        )
                    st2t = s2.tile([128, T2C], BF16, tag="st2")
                    nc.scalar.activation(st2t[:], ps2t[:], AF.Copy)
                    nc.sync.dma_start(t2slice[b], st2t[:])
                    nc.sync.dma_start(
                        sdst_dram[b * 128:(b + 1) * 128, :], st2t[:, 257:258]
                    )

            nc.gpsimd.collective_compute(
                "AllGather",
                mybir.AluOpType.bypass,
                replica_groups=[list(range(N_CORES))],
                ins=[t2slice[:, :, :]],
                outs=[t2full[:, :, :, :]],
            )

            # ---------------- phase 2B: GAT dense push -----------------------
            with (
                tc.tile_pool(name="t2_p", bufs=1) as tp2,
                tc.tile_pool(name="row_p", bufs=1) as rp,
                tc.tile_pool(name="bc_p", bufs=1) as bcp,
                tc.tile_pool(name="b2g_p", bufs=4) as bp,
                tc.tile_pool(name="T_p", bufs=2) as Tp,
                tc.tile_pool(name="H_p", bufs=2) as Hp,
                tc.tile_pool(name="L_p", bufs=2) as Lp,
                tc.tile_pool(name="A_p", bufs=3) as Ap,
                tc.psum_pool(name="ps2b_p", bufs=1) as pb,
                tc.tile_pool(name="fin_p", bufs=2) as fp_,
            ):
                t2_sb = tp2.tile([128, ngb, T2C], BF16)
                # fp32 copy of the ssrc logit columns (tensor_scalar needs
                # fp32 per-partition scalars)
                ssrc_f32 = bcp.tile([128, ngb], FP32)
                for r in range(N_CORES):
                    nc.sync.dma_start(
                        t2_sb[:, r * nblk:(r + 1) * nblk, :],
                        t2full[r].rearrange("b p c -> p b c"),
                    )
                    nc.vector.tensor_copy(
                        ssrc_f32[:, r * nblk:(r + 1) * nblk],
                        t2_sb[:, r * nblk:(r + 1) * nblk, 256:257],
                    )
                sdstrow = rp.tile([1, per], BF16)
                nc.sync.dma_start(sdstrow[:], sdst_dram[:, :])
                sdst_bc = bcp.tile([128, per], BF16)
                nc.gpsimd.partition_broadcast(sdst_bc[:], sdstrow[:])

                PA = pb.tile([128, 512], FP32)
                PB = pb.tile([128, 512], FP32)
                PD = pb.tile([128, 512], FP32)
                PE_ = pb.tile([128, 512], FP32)
                PC_ = pb.tile([128, 512], FP32)
                DN0 = pb.tile([128, 512], FP32)
                DN1 = pb.tile([128, 512], FP32)
                DN2 = pb.tile([128, 512], FP32)

                def t2_dst(h, ci):
                    if ci < 2:
                        t = (PA, PB)[ci] if h == 0 else (PD, PE_)[ci]
                        return t[:, :]
                    return PC_[:, 0:256] if h == 0 else PC_[:, 256:512]

                dn = [DN0[0:1, :], DN1[0:1, :], DN2[0:1, 0:256]]

                for g in range(ngb):
                    bg = bp.tile([128, per], BF16, tag="b2g")
                    nc.sync.dma_start(bg[:], b2s_d[g])
                    # T = ssrc_g + sdst ; L = leakyrelu(T) = max(T, 0.2T)
                    # L2 = L + (log(count) - C | -inf) ; A = exp(L2)
                    Tt = Tp.tile([128, per], BF16, tag="T")
                    nc.vector.tensor_scalar_add(
                        Tt[:], sdst_bc[:], ssrc_f32[:, g:g + 1]
                    )
                    Ht = Hp.tile([128, per], BF16, tag="H")
                    nc.vector.tensor_scalar_mul(Ht[:], Tt[:], 0.2)
                    Lt = Lp.tile([128, per], BF16, tag="L")
                    nc.vector.tensor_tensor(Lt[:], Tt[:], Ht[:], op=MAX)
                    L2 = Lp.tile([128, per], BF16, tag="L2")
                    nc.vector.tensor_tensor(L2[:], Lt[:], bg[:], op=ADD)
                    At = Ap.tile([128, per], BF16, tag="A")
                    nc.scalar.activation(At[:], L2[:], AF.Exp)
                    st, sp = (g == 0), (g == ngb - 1)
                    for h in range(2):
                        for ci, (c0, c1) in enumerate(CH):
                            # PC_ bank shared by h0/h1 chunk-2 groups: only
                            # the h0 group may issue the bank-clearing start.
                            nc.tensor.matmul(
                                t2_dst(h, ci),
                                lhsT=t2_sb[:, g, h * 128:(h + 1) * 128],
                                rhs=At[:, c0:c1],
                                start=st and not (h == 1 and ci == 2),
                                stop=sp,
                            )
                    for ci, (c0, c1) in enumerate(CH):
                        nc.tensor.matmul(
                            dn[ci],
                            lhsT=ones_sb[:],
                            rhs=At[:, c0:c1],
                            start=st,
                            stop=sp,
                        )

                # ---- normalize + bias + relu, write transposed --------------
                denrow = rp.tile([1, per], FP32)
                for ci, (c0, c1) in enumerate(CH):
                    nc.vector.tensor_copy(denrow[:, c0:c1], dn[ci])
                den_bc = bcp.tile([128, per], FP32)
                nc.gpsimd.partition_broadcast(den_bc[:], denrow[:])
                rden = bcp.tile([128, per], FP32)
                nc.vector.reciprocal(rden[:], den_bc[:])

                for h in range(2):
                    for ci, (c0, c1) in enumerate(CH):
                        csz = c1 - c0
                        tmp = fp_.tile([128, 512], FP32, tag="tmp")
                        nc.vector.tensor_tensor(
                            tmp[:, 0:csz], t2_dst(h, ci), rden[:, c0:c1],
                            op=MUL,
                        )
                        oc = fp_.tile([128, 512], FP32, tag="oc")
                        nc.vector.tensor_scalar(
                            oc[:, 0:csz], tmp[:, 0:csz], b2_sb[:, h:h + 1],
                            0.0, op0=ADD, op1=MAX,
                        )
                        nc.sync.dma_start(outT_d[h, :, c0:c1], oc[:, 0:csz])

    nc.finalize()
    return nc


# ----------------------------------------------------------------------------
# entry point
# ----------------------------------------------------------------------------

_CACHE = {}


def _get_nc(npad):
    if npad not in _CACHE:
        _CACHE[npad] = _build_nc(npad)
    return _CACHE[npad]


def kernel(event_emb, edge_index, W1, b1, W2, att_src, att_dst, b2,
           _want_results=False, _trace=False):
    shared, per_core, n, npad, per, ngb = _prep(
        event_emb, edge_index, W1, b1, W2, att_src, att_dst, b2
    )
    nc = _get_nc(npad)
    in_maps = [{**shared, **per_core[c]} for c in range(N_CORES)]
    res = run_bass_kernel_spmd(
        nc, in_maps, core_ids=list(range(N_CORES)), trace=_trace
    )
    outs = []
    for c in range(N_CORES):
        oT = np.asarray(res.results[c]["outT"], np.float32)  # [2,128,per]
        outs.append(oT.reshape(D, per).T)  # [per, D]
    out = np.concatenate(outs, axis=0)[:n]
    if _want_results:
        return out, res
    return out


# revision 27
# speedup vs baseline: 1.1564x; 1.0804x over previous
"""Trainium2 Bass kernel for EventDiffusion GNN (GCNConv + GATConv, 2 layers).

Dense block-push formulation (no per-edge gathers, no Q7 descriptor storms):

  - nodes padded to NPAD=10240; dst range sharded 8 ways (1280 dst/core);
    src dimension is global (10240) on every core.
  - Layer 1 (GCN): per src-block g (80 blocks of 128), compute
    xw1_g = X_g @ W1 on the fly, then push
       psumT[feat, dst_local] += xw1_g^T @ m1_g
    where m1_g[src_slot, dst_local] is a host-precomputed dense bf16 matrix of
    summed GCN coefficients (zero where no edge).  Output lands transposed
    (H^T), which is exactly the lhsT layout needed by layer 2.
  - Layer 2 (GAT): attention logits are separable: e[s,d] =
    leakyrelu(ssrc[s] + sdst[d]) for (s,d) edges.  Per src block:
       T = (B_g + ssrc_g) + sdst_bcast          (B_g = log(edge count), -3e4 if none)
       A = exp(max(T, 0.2T) - C)                (C=10 constant shift; cancels in softmax)
       psumT[feat, dst] += t2_g^T @ A ; den += ones^T @ A
    Multi-edges are handled exactly: exp(log(count) + e) = count * exp(e).
  - softmax normalization: out = relu(psumT * (1/den) + b2), written transposed;
    host transposes back.
  - one AllGather of the 260-col layer-2 node table (features + ssrc + sdst).
"""

import numpy as np

import concourse.bass as bass
import concourse.bacc as bacc
import concourse.mybir as mybir
import concourse.tile as tile
from concourse.bass_utils import run_bass_kernel_spmd
from concourse import dve_ops as _dve_ops
from concourse.dve_spec import Spec as _Spec, Src0 as _Src0, Src1 as _Src1, \
    C0 as _C0, C1 as _C1, maxx as _maxx


def _register_gat_logit_op():
    """Fused DVE op: out = leakyrelu(in0 + s0, slope=s1) + in1
    (one 1x-rate pass instead of a 4-instruction chain)."""
    name = "GAT_LOGIT_FUSED"
    if name in _dve_ops._SUB_OPCODE_FOR_NAME:
        return next(o for o in _dve_ops.OPS if o.name == name)
    u = _Src0 + _C0
    op = _dve_ops.DveOp(
        name,
        _Spec(
            body=_maxx(u, u * _C1) + _Src1,
            reference=lambda in0, in1, s0, s1, imm2: (
                np.maximum(in0 + s0, (in0 + s0) * s1) + in1
            ).astype(np.float32),
        ),
        subdim=False,
        uops_sha={"v3": "a37ff0300d9eb99c", "v4": "72a01b2685921ef2"},
    )
    _dve_ops.OPS.append(op)
    _dve_ops.CUSTOM_DVE_SPECS[op.name] = op.spec
    _dve_ops._SUB_OPCODE_FOR_NAME[op.name] = (
        max(_dve_ops._SUB_OPCODE_FOR_NAME.values()) + 1
    )
    return op


_GAT_LOGIT_FUSED = _register_gat_logit_op()

FP32 = mybir.dt.float32
BF16 = mybir.dt.bfloat16
BF16NP = mybir.dt.np(mybir.dt.bfloat16)

N_CORES = 8
D = 256
T2C = 260          # layer-2 table cols: 256 feats | 256 ssrc | 257 sdst | pad
CSHIFT = 0.0       # constant softmax shift (cancels exactly in the ratio);
                   # logits for this distribution are <1, exp overflows only
                   # past ~85, and a nonzero shift stored in bf16 would cost
                   # ~1.6% relative noise on every attention weight
NEGINF = -30000.0  # log-count placeholder for non-edges

ADD = mybir.AluOpType.add
MUL = mybir.AluOpType.mult
MAX = mybir.AluOpType.max
AF = mybir.ActivationFunctionType


def _bf16(a):
    return np.ascontiguousarray(np.asarray(a, np.float32)).astype(BF16NP)


def _pad_nodes(n):
    return -(-n // (128 * N_CORES)) * (128 * N_CORES)


# ----------------------------------------------------------------------------
# host-side preprocessing
# ----------------------------------------------------------------------------

def _prep(event_emb, edge_index, W1, b1, W2, att_src, att_dst, b2):
    X = np.asarray(event_emb, np.float32)
    n = X.shape[0]
    npad = _pad_nodes(n)
    per = npad // N_CORES
    ngb = npad // 128

    ei = np.asarray(edge_index, np.int64)
    src = np.concatenate([ei[0], np.arange(n, dtype=np.int64)])
    dst = np.concatenate([ei[1], np.arange(n, dtype=np.int64)])
    deg = np.bincount(dst, minlength=n).astype(np.float32)
    dinv = np.where(deg > 0, 1.0 / np.sqrt(deg), 0.0).astype(np.float32)
    coeff = (dinv[src] * dinv[dst]).astype(np.float32)

    core_of = dst // per
    per_core = []
    for c in range(N_CORES):
        m = core_of == c
        s, d = src[m], dst[m] - c * per
        co = coeff[m]
        flat = s * per + d
        m1 = np.zeros(npad * per, np.float32)
        np.add.at(m1, flat, co)
        cnt = np.zeros(npad * per, np.float32)
        np.add.at(cnt, flat, 1.0)
        b2m = np.full(npad * per, NEGINF, np.float32)
        nz = cnt > 0
        b2m[nz] = np.log(cnt[nz]) - CSHIFT
        per_core.append(
            dict(
                m1s=_bf16(m1.reshape(ngb, 128, per)),
                b2s=_bf16(b2m.reshape(ngb, 128, per)),
            )
        )
        del m1, cnt, b2m

    W1 = np.asarray(W1, np.float32)
    W2 = np.asarray(W2, np.float32)
    v1 = (W2 @ np.asarray(att_src, np.float32)).astype(np.float32)
    v2 = (W2 @ np.asarray(att_dst, np.float32)).astype(np.float32)

    Xp = np.zeros((npad, D), np.float32)
    Xp[:n] = X
    W2p = np.zeros((D, T2C), np.float32)
    W2p[:, :D] = W2
    W2p[:, 256] = v1
    W2p[:, 257] = v2

    shared = dict(
        xtb=_bf16(Xp.T.reshape(2, 128, npad)),
        w1b=_bf16(W1.reshape(2, 128, D)),
        w2p=_bf16(W2p.reshape(2, 128, T2C)),
        b1T=np.ascontiguousarray(np.asarray(b1, np.float32).reshape(2, 128).T),
        b2T=np.ascontiguousarray(np.asarray(b2, np.float32).reshape(2, 128).T),
        ones128=_bf16(np.ones((128, 1), np.float32)),
    )
    return shared, per_core, n, npad, per, ngb


# ----------------------------------------------------------------------------
# device program
# ----------------------------------------------------------------------------

def _build_nc(npad):
    per = npad // N_CORES
    ngb = npad // 128
    nblk = per // 128
    # dst column chunks per feature half: psum banks are 512 fp32 wide
    CH = [(0, 512), (512, 1024), (1024, 1280)]
    assert per == 1280

    nc = bacc.Bacc(
        "TRN2", target_bir_lowering=False, debug=False, num_devices=N_CORES
    )

    xtb_d = nc.dram_tensor("xtb", [2, 128, npad], BF16, kind="ExternalInput")
    w1_d = nc.dram_tensor("w1b", [2, 128, D], BF16, kind="ExternalInput")
    w2_d = nc.dram_tensor("w2p", [2, 128, T2C], BF16, kind="ExternalInput")
    b1_d = nc.dram_tensor("b1T", [128, 2], FP32, kind="ExternalInput")
    b2_d = nc.dram_tensor("b2T", [128, 2], FP32, kind="ExternalInput")
    ones_d = nc.dram_tensor("ones128", [128, 1], BF16, kind="ExternalInput")
    m1_d = nc.dram_tensor("m1s", [ngb, 128, per], BF16, kind="ExternalInput")
    b2s_d = nc.dram_tensor("b2s", [ngb, 128, per], BF16, kind="ExternalInput")
    outT_d = nc.dram_tensor("outT", [2, 128, per], FP32, kind="ExternalOutput")

    t2slice = nc.dram_tensor("t2slice", [nblk, 128, T2C], BF16)
    sdst_dram = nc.dram_tensor("sdstd", [per, 1], BF16)
    t2full = nc.dram_tensor(
        "t2full", [N_CORES, nblk, 128, T2C], BF16, addr_space="Shared"
    )

    with tile.TileContext(nc) as tc:
        with tc.tile_pool(name="const", bufs=1) as cp:
            w1_sb = cp.tile([128, 2, D], BF16)
            w2_sb = cp.tile([128, 2, T2C], BF16)
            for k in range(2):
                nc.sync.dma_start(w1_sb[:, k, :], w1_d[k])
                nc.sync.dma_start(w2_sb[:, k, :], w2_d[k])
            b1_sb = cp.tile([128, 2], FP32)
            nc.sync.dma_start(b1_sb[:], b1_d[:, :])
            b2_sb = cp.tile([128, 2], FP32)
            nc.sync.dma_start(b2_sb[:], b2_d[:, :])
            ones_sb = cp.tile([128, 1], BF16)
            nc.sync.dma_start(ones_sb[:], ones_d[:, :])
            ht_sb = cp.tile([128, 2, per], BF16)

            # ---------------- phase 1: GCN (fused XW1 + push) ----------------
            with (
                tc.tile_pool(name="xt_p", bufs=1) as xp,
                tc.tile_pool(name="m1_p", bufs=3) as mp,
                tc.tile_pool(name="xw1_p", bufs=4) as wp,
                tc.psum_pool(name="ps1a_p", bufs=2) as pa,
                tc.psum_pool(name="psT1_p", bufs=1) as pt,
            ):
                xt_sb = xp.tile([128, 2, npad], BF16)
                for k in range(2):
                    nc.sync.dma_start(xt_sb[:, k, :], xtb_d[k])
                # psumT tiles: h0 -> TA,TB,TC[:, :256]; h1 -> TD,TE,TC[:,256:]
                TA = pt.tile([128, 512], FP32)
                TB = pt.tile([128, 512], FP32)
                TD = pt.tile([128, 512], FP32)
                TE = pt.tile([128, 512], FP32)
                TC_ = pt.tile([128, 512], FP32)

                def t1_dst(h, ci):
                    if ci < 2:
                        t = (TA, TB)[ci] if h == 0 else (TD, TE)[ci]
                        return t[:, :]
                    return TC_[:, 0:256] if h == 0 else TC_[:, 256:512]

                for g in range(ngb):
                    m1g = mp.tile([128, per], BF16, tag="m1")
                    nc.sync.dma_start(m1g[:], m1_d[g])
                    ps = pa.tile([128, D], FP32, tag="ps1a")
                    for k in range(2):
                        nc.tensor.matmul(
                            ps[:],
                            lhsT=xt_sb[:, k, g * 128:(g + 1) * 128],
                            rhs=w1_sb[:, k, :],
                            start=(k == 0),
                            stop=(k == 1),
                        )
                    xg = wp.tile([128, D], BF16, tag="xw1")
                    nc.scalar.activation(xg[:], ps[:], AF.Copy)
                    st, sp = (g == 0), (g == ngb - 1)
                    for h in range(2):
                        for ci, (c0, c1) in enumerate(CH):
                            # TC_ holds two accumulation groups in one PSUM
                            # bank; start=True clears the WHOLE bank, so only
                            # the first-issued group (h0) may set it.  The h1
                            # group overwrites its freshly-cleared region via
                            # the per-element has_written bits.
                            nc.tensor.matmul(
                                t1_dst(h, ci),
                                lhsT=xg[:, h * 128:(h + 1) * 128],
                                rhs=m1g[:, c0:c1],
                                start=st and not (h == 1 and ci == 2),
                                stop=sp,
                            )
                # H = relu(aggT + b1), stored transposed bf16
                for h in range(2):
                    for ci, (c0, c1) in enumerate(CH):
                        nc.vector.tensor_scalar(
                            ht_sb[:, h, c0:c1],
                            t1_dst(h, ci),
                            b1_sb[:, h:h + 1],
                            0.0,
                            op0=ADD,
                            op1=MAX,
                        )

            # ---------------- phase 2A: local table2 slice -------------------
            with (
                tc.psum_pool(name="ps2_p", bufs=2) as p2,
                tc.tile_pool(name="st2_p", bufs=3) as s2,
            ):
                for b in range(nblk):
                    ps2t = p2.tile([128, T2C], FP32, tag="ps2")
                    for k in range(2):
                        nc.tensor.matmul(
                            ps2t[:],
                            lhsT=ht_sb[:, k, b * 128:(b + 1) * 128],
                            rhs=w2_sb[:, k, :],
                            start=(k == 0),
                            stop=(k == 1),
                        )
                    st2t = s2.tile([128, T2C], BF16, tag="st2")
                    nc.scalar.activation(st2t[:], ps2t[:], AF.Copy)
                    nc.sync.dma_start(t2slice[b], st2t[:])
                    nc.sync.dma_start(
                        sdst_dram[b * 128:(b + 1) * 128, :], st2t[:, 257:258]
                    )

            nc.gpsimd.collective_compute(
                "AllGather",
                mybir.AluOpType.bypass,
                replica_groups=[list(range(N_CORES))],
                ins=[t2slice[:, :, :]],
                outs=[t2full[:, :, :, :]],
            )

            # ---------------- phase 2B: GAT dense push -----------------------
            with (
                tc.tile_pool(name="t2_p", bufs=1) as tp2,
                tc.tile_pool(name="row_p", bufs=1) as rp,
                tc.tile_pool(name="bc_p", bufs=1) as bcp,
                tc.tile_pool(name="b2g_p", bufs=4) as bp,
                tc.tile_pool(name="T_p", bufs=2) as Tp,
                tc.tile_pool(name="H_p", bufs=2) as Hp,
                tc.tile_pool(name="L_p", bufs=2) as Lp,
                tc.tile_pool(name="A_p", bufs=3) as Ap,
                tc.psum_pool(name="ps2b_p", bufs=1) as pb,
                tc.tile_pool(name="fin_p", bufs=2) as fp_,
            ):
                t2_sb = tp2.tile([128, ngb, T2C], BF16)
                # fp32 copy of the ssrc logit columns (tensor_scalar needs
                # fp32 per-partition scalars)
                ssrc_f32 = bcp.tile([128, ngb], FP32)
                for r in range(N_CORES):
                    nc.sync.dma_start(
                        t2_sb[:, r * nblk:(r + 1) * nblk, :],
                        t2full[r].rearrange("b p c -> p b c"),
                    )
                    nc.vector.tensor_copy(
                        ssrc_f32[:, r * nblk:(r + 1) * nblk],
                        t2_sb[:, r * nblk:(r + 1) * nblk, 256:257],
                    )
                sdstrow = rp.tile([1, per], BF16)
                nc.sync.dma_start(sdstrow[:], sdst_dram[:, :])
                sdst_bc = bcp.tile([128, per], BF16)
                nc.gpsimd.partition_broadcast(sdst_bc[:], sdstrow[:])

                PA = pb.tile([128, 512], FP32)
                PB = pb.tile([128, 512], FP32)
                PD = pb.tile([128, 512], FP32)
                PE_ = pb.tile([128, 512], FP32)
                PC_ = pb.tile([128, 512], FP32)
                DN0 = pb.tile([128, 512], FP32)
                DN1 = pb.tile([128, 512], FP32)
                DN2 = pb.tile([128, 512], FP32)

                def t2_dst(h, ci):
                    if ci < 2:
                        t = (PA, PB)[ci] if h == 0 else (PD, PE_)[ci]
                        return t[:, :]
                    return PC_[:, 0:256] if h == 0 else PC_[:, 256:512]

                dn = [DN0[0:1, :], DN1[0:1, :], DN2[0:1, 0:256]]

                for g in range(ngb):
                    bg = bp.tile([128, per], BF16, tag="b2g")
                    nc.sync.dma_start(bg[:], b2s_d[g])
                    # T = ssrc_g + sdst ; L = leakyrelu(T) = max(T, 0.2T)
                    # L2 = L + (log(count) - C | -inf) ; A = exp(L2)
                    # L2 = leakyrelu(ssrc + sdst) + logcount  in ONE DVE pass
                    L2 = Lp.tile([128, per], BF16, tag="L2")
                    nc.vector._custom_dve(
                        _GAT_LOGIT_FUSED,
                        out=L2[:],
                        in0=sdst_bc[:],
                        in1=bg[:],
                        s0=ssrc_f32[:, g:g + 1],
                        s1=0.2,
                    )
                    At = Ap.tile([128, per], BF16, tag="A")
                    nc.scalar.activation(At[:], L2[:], AF.Exp)
                    st, sp = (g == 0), (g == ngb - 1)
                    for h in range(2):
                        for ci, (c0, c1) in enumerate(CH):
                            # PC_ bank shared by h0/h1 chunk-2 groups: only
                            # the h0 group may issue the bank-clearing start.
                            nc.tensor.matmul(
                                t2_dst(h, ci),
                                lhsT=t2_sb[:, g, h * 128:(h + 1) * 128],
                                rhs=At[:, c0:c1],
                                start=st and not (h == 1 and ci == 2),
                                stop=sp,
                            )
                    for ci, (c0, c1) in enumerate(CH):
                        nc.tensor.matmul(
                            dn[ci],
                            lhsT=ones_sb[:],
                            rhs=At[:, c0:c1],
                            start=st,
                            stop=sp,
                        )

                # ---- normalize + bias + relu, write transposed --------------
                denrow = rp.tile([1, per], FP32)
                for ci, (c0, c1) in enumerate(CH):
                    nc.vector.tensor_copy(denrow[:, c0:c1], dn[ci])
                den_bc = bcp.tile([128, per], FP32)
                nc.gpsimd.partition_broadcast(den_bc[:], denrow[:])
                rden = bcp.tile([128, per], FP32)
                nc.vector.reciprocal(rden[:], den_bc[:])

                for h in range(2):
                    for ci, (c0, c1) in enumerate(CH):
                        csz = c1 - c0
                        tmp = fp_.tile([128, 512], FP32, tag="tmp")
                        nc.vector.tensor_tensor(
                            tmp[:, 0:csz], t2_dst(h, ci), rden[:, c0:c1],
                            op=MUL,
                        )
                        oc = fp_.tile([128, 512], FP32, tag="oc")
                        nc.vector.tensor_scalar(
                            oc[:, 0:csz], tmp[:, 0:csz], b2_sb[:, h:h + 1],
                            0.0, op0=ADD, op1=MAX,
                        )
                        nc.sync.dma_start(outT_d[h, :, c0:c1], oc[:, 0:csz])

    nc.finalize()
    return nc


# ----------------------------------------------------------------------------
# entry point
# ----------------------------------------------------------------------------

_CACHE = {}


def _get_nc(npad):
    if npad not in _CACHE:
        _CACHE[npad] = _build_nc(npad)
    return _CACHE[npad]


def kernel(event_emb, edge_index, W1, b1, W2, att_src, att_dst, b2,
           _want_results=False, _trace=False):
    shared, per_core, n, npad, per, ngb = _prep(
        event_emb, edge_index, W1, b1, W2, att_src, att_dst, b2
    )
    nc = _get_nc(npad)
    in_maps = [{**shared, **per_core[c]} for c in range(N_CORES)]
    res = run_bass_kernel_spmd(
        nc, in_maps, core_ids=list(range(N_CORES)), trace=_trace
    )
    outs = []
    for c in range(N_CORES):
        oT = np.asarray(res.results[c]["outT"], np.float32)  # [2,128,per]
        outs.append(oT.reshape(D, per).T)  # [per, D]
    out = np.concatenate(outs, axis=0)[:n]
    if _want_results:
        return out, res
    return out
